# revision 17
# baseline (speedup 1.0000x reference)
"""Masked dot-product attention (B=64, S=1024, D=64) on 8 Trainium2 NeuronCores.

Strategy (per core, 8 batches):
  - Compute S^T chunks [k=128, q=1024] = K_chunk @ Q^T via PE matmuls with the
    D=64 contraction on partitions; two chunks run concurrently in the two
    64-row strips of the PE array (tile_position row packing).
  - exp via ACT with per-partition bias = 0 / -30000 -> masked probabilities
    P^T in fp16 (mask folded into the softmax for free; scale 1/sqrt(D) folded
    into the ACT scale).
  - P @ [V | 1] via PE matmuls with P^T slices stationary; column 64 of the
    accumulator is the softmax denominator.
  - normalize with reciprocal + per-partition tensor_scalar_mul, DMA out.
Host does layout prep only (transpose/cast/shard) - all FLOPs on device.
"""

import numpy as np

import concourse.bass as bass  # noqa: F401  (registers engine types)
import concourse.bacc as bacc
import concourse.mybir as mybir
import concourse.tile as tile
from concourse.bass_utils import run_bass_kernel_spmd

B, S, D = 64, 1024, 64
NCORES = 8
BPC = B // NCORES          # batches per core
NCH = S // 128             # k chunks of 128
NQT = S // 128             # q tiles of 128
F16 = mybir.dt.float16
F32 = mybir.dt.float32

_NC_CACHE = {}


def _build_nc(loop_reps=None, slot_counts=(NCH,) * BPC):
    nc = bacc.Bacc(None, target_bir_lowering=False)
    qtd = nc.dram_tensor("qtd", [BPC, 128, S], F16, kind="ExternalInput")
    ktp = nc.dram_tensor("ktp", [BPC, 128, NCH // 2, 128], F16, kind="ExternalInput")
    vp = nc.dram_tensor("vp", [BPC, 128, NCH, D + 1], F16, kind="ExternalInput")
    bia = nc.dram_tensor("bia", [BPC, 128, NCH], F32, kind="ExternalInput")
    ot = nc.dram_tensor("ot", [BPC, 128, NQT, D], F32, kind="ExternalOutput")

    import contextlib

    with tile.TileContext(nc) as tc:
        with (
            tc.tile_pool(name="qpool", bufs=3) as qpool,
            tc.tile_pool(name="kpool", bufs=3) as kpool,
            tc.tile_pool(name="vpool", bufs=3) as vpool,
            tc.tile_pool(name="bpool", bufs=3) as bpool,
            tc.tile_pool(name="ppool", bufs=10) as ppool,
            tc.tile_pool(name="outpool", bufs=2) as outpool,
            tc.tile_pool(name="rpool", bufs=4) as rpool,
            tc.tile_pool(name="spool", bufs=3, space="PSUM") as spool,
            tc.tile_pool(name="accpool", bufs=1, space="PSUM") as accpool,
            tc.For_i(0, loop_reps, 1) if loop_reps else contextlib.nullcontext(),
        ):
            for b in range(BPC):
                n = max(1, min(NCH, slot_counts[b]))  # k-chunks to compute
                npair = (n + 1) // 2
                qt = qpool.tile([128, S], F16)
                nc.sync.dma_start(out=qt, in_=qtd.ap()[b])
                kt = kpool.tile([128, NCH // 2, 128], F16, tag="kt")
                nc.sync.dma_start(
                    out=kt[:, :npair, :], in_=ktp.ap()[b][:, :npair, :]
                )
                vv = vpool.tile([128, NCH, D + 1], F16, tag="vv")
                nc.sync.dma_start(out=vv[:, :n, :], in_=vp.ap()[b][:, :n, :])
                bi = bpool.tile([128, NCH], F32, tag="bi")
                nc.sync.dma_start(out=bi[:, :n], in_=bia.ap()[b][:, :n])

                acc0 = accpool.tile([128, 4, D + 1], F32, tag="acc0")
                acc1 = accpool.tile([128, 4, D + 1], F32, tag="acc1")

                pms = []
                for j in range(npair):
                    st0 = spool.tile([128, S], F32, tag="st")
                    st1 = (
                        spool.tile([128, S], F32, tag="st", name="st1")
                        if 2 * j + 1 < n else None
                    )
                    for h in range(2):
                        nc.tensor.matmul(
                            st0[:, h * 512:(h + 1) * 512],
                            lhsT=kt[0:64, j, :],
                            rhs=qt[0:64, h * 512:(h + 1) * 512],
                            start=True, stop=True,
                        )
                        if st1 is not None:
                            nc.tensor.matmul(
                                st1[:, h * 512:(h + 1) * 512],
                                lhsT=kt[64:128, j, :],
                                rhs=qt[64:128, h * 512:(h + 1) * 512],
                                start=True, stop=True,
                            )
                    pairs = [(2 * j, st0)]
                    if st1 is not None:
                        pairs.append((2 * j + 1, st1))
                    for cc, st in pairs:
                        pm = ppool.tile([128, S], F16, tag="pm")
                        nc.scalar.activation(
                            out=pm, in_=st,
                            func=mybir.ActivationFunctionType.Exp,
                            bias=bi[:, cc:cc + 1], scale=0.125,
                        )
                        pms.append(pm)
                # PV: one sequential accumulation group per q-tile slice
                for t in range(NQT):
                    acc = acc0 if t < 4 else acc1
                    for cc in range(n):
                        nc.tensor.matmul(
                            acc[:, t % 4, :],
                            lhsT=pms[cc][:, t * 128:(t + 1) * 128],
                            rhs=vv[:, cc, :],
                            start=(cc == 0), stop=(cc == n - 1),
                        )

                osb = outpool.tile([128, NQT, D], F32)
                for half, acc in ((0, acc0), (1, acc1)):
                    r = rpool.tile([128, 4], F32, tag="r")
                    nc.vector.tensor_scalar_add(out=r, in0=acc[:, :, D], scalar1=1e-30)
                    nc.vector.reciprocal(r, r)
                    for t4 in range(4):
                        t = half * 4 + t4
                        nc.vector.tensor_scalar_mul(
                            out=osb[:, t, :],
                            in0=acc[:, t4, 0:D],
                            scalar1=r[:, t4:t4 + 1],
                        )
                nc.sync.dma_start(out=ot.ap()[b], in_=osb)

    nc.compile()
    return nc


def _get_nc(slot_counts=(NCH,) * BPC):
    key = tuple(slot_counts)
    if key not in _NC_CACHE:
        _NC_CACHE[key] = _build_nc(slot_counts=key)
    return _NC_CACHE[key]


def kernel(queries, keys, values, valid_lens):
    queries = np.asarray(queries, dtype=np.float32)
    keys = np.asarray(keys, dtype=np.float32)
    values = np.asarray(values, dtype=np.float32)
    lens = np.asarray(valid_lens).astype(np.int64)

    q16 = queries.astype(np.float16)
    k16 = keys.astype(np.float16)
    v16 = values.astype(np.float16)

    # Q^T duplicated across both 64-partition strips: [B, 128, S]
    qt = np.ascontiguousarray(q16.transpose(0, 2, 1))
    qtd = np.concatenate([qt, qt], axis=1)

    # K^T packed: partitions 0-63 hold even k-chunks, 64-127 odd: [B, 128, 4, 128]
    kt4 = k16.transpose(0, 2, 1).reshape(B, 64, NCH, 128)
    ktp = np.ascontiguousarray(
        np.concatenate([kt4[:, :, 0::2, :], kt4[:, :, 1::2, :]], axis=1)
    )

    # V with appended ones column, chunk-major on partitions: [B, 128, NCH, D+1]
    vp = np.empty((B, 128, NCH, D + 1), np.float16)
    vp[:, :, :, :D] = v16.reshape(B, NCH, 128, D).transpose(0, 2, 1, 3)
    vp[:, :, :, D] = np.float16(1.0)

    # exp bias: 0 where k position valid, -30000 where masked: [B, 128, NCH]
    kpos = np.arange(S).reshape(NCH, 128).T  # [128, NCH] -> k = c*128 + p
    bia = np.where(kpos[None] < lens[:, None, None], 0.0, -30000.0).astype(np.float32)

    # Length specialization: batch i needs ceil(L_i/128) k-chunks (min 1).
    # Sort by need and deal round-robin so every core's slot s holds batches
    # with (near-)equal chunk counts; slot count = max within the deal group,
    # so all cores run the identical compiled program perfectly balanced.
    need = np.maximum(1, -(-lens // 128)).astype(np.int64)  # ceil, >=1
    order = np.argsort(need, kind="stable")  # batch ids, ascending need
    gmax = [
        int(need[order[g * NCORES:(g + 1) * NCORES]].max()) for g in range(BPC)
    ]
    # emit large/small groups interleaved -- keeps engines busier at the
    # seams (measured ~3% in the cost model)
    perm = [7, 0, 6, 1, 5, 2, 4, 3][:BPC]
    slot_counts = tuple(gmax[p] for p in perm)
    # core c, slot s runs original batch order[perm[s]*NCORES + c]
    nc = _get_nc(slot_counts)
    in_maps = []
    for c in range(NCORES):
        ids = [int(order[perm[s] * NCORES + c]) for s in range(BPC)]
        in_maps.append({
            "qtd": np.ascontiguousarray(qtd[ids]),
            "ktp": np.ascontiguousarray(ktp[ids]),
            "vp": np.ascontiguousarray(vp[ids]),
            "bia": np.ascontiguousarray(bia[ids]),
        })

    res = run_bass_kernel_spmd(nc, in_maps, core_ids=list(range(NCORES)))

    out = np.empty((B, S, D), np.float32)
    for c in range(NCORES):
        otv = res.results[c]["ot"]  # [BPC, 128, NQT, D]
        ids = [int(order[perm[s] * NCORES + c]) for s in range(BPC)]
        out[ids] = otv.transpose(0, 2, 1, 3).reshape(BPC, S, D)
    return out


# revision 35
# speedup vs baseline: 1.3566x; 1.3566x over previous
"""Masked dot-product attention (B=64, S=1024, D=64) on 8 Trainium2 NeuronCores.

Strategy (per core, 8 batches, valid-length-specialized to n k-chunks/batch):
  - One fused input DMA per batch: [Qhalf | Kt x2 | V' | bias] packed per
    partition (DMA latency is 650ns/issue; traffic is the measured floor).
  - S^T chunks [k=128, q=1024] = K_chunk @ Q^T on PE, D=64 contraction on
    partitions; the two 64-row strips of the PE array compute the two q-halves
    of the SAME chunk concurrently (tile_position row packing, no Q dup).
  - exp via ACT, per-partition bias 0/-30000 folds the valid_lens mask into
    the softmax; 1/sqrt(D) folded into the ACT scale. P^T in fp16.
  - P @ [V | 1]: P^T slices stationary; column 64 of the accumulator is the
    softmax denominator. normalize = reciprocal + tensor_scalar_mul -> fp16
    out, upcast to fp32 on host.
Host does layout prep only (transpose/cast/pack/shard) - all FLOPs on device.
"""

import contextlib

import numpy as np

import concourse.bass as bass  # noqa: F401
import concourse.bacc as bacc
import concourse.mybir as mybir
import concourse.tile as tile
from concourse.bass_utils import run_bass_kernel_spmd

B, S, D = 64, 1024, 64
NCORES = 8
BPC = B // NCORES          # batches per core
NCH = S // 128             # k chunks of 128
NQT = S // 128             # q tiles of 128
F16 = mybir.dt.float16
F32 = mybir.dt.float32

# fused input row layout (f16 elements per partition):
#   [0:512)                q half (strip 0: q 0-511, strip 1: q 512-1023)
#   [512 : 512+128n)       K^T chunk c at 512+128c (same data in both strips)
#   [+ : +66n)             V' chunk c at +66c (65 used + 1 pad)
#   [+ : +n)               exp bias per chunk (0 / -30000, f16)
ROW = 512 + 195 * NCH  # 2072

_NC_CACHE = {}


def _build_nc(loop_reps=None, slot_counts=(NCH,) * BPC, ablate=frozenset()):
    # slot_counts entries: n (chunks to compute) or (n, nz) where chunks
    # 0..nz-2 are fully valid for EVERY batch dealt into that slot (group min
    # need) and may share a fused zero-bias exp; the rest always read their
    # per-chunk bias vector.
    nc = bacc.Bacc(None, target_bir_lowering=False)
    inp = nc.dram_tensor("inp", [BPC, 128, ROW], F16, kind="ExternalInput")
    ot = nc.dram_tensor("ot", [BPC, 128, NQT, D], F16, kind="ExternalOutput")

    with tile.TileContext(nc) as tc:
        with (
            tc.tile_pool(name="inpool", bufs=3) as inpool,
            tc.tile_pool(name="ppool", bufs=18) as ppool,
            tc.tile_pool(name="outpool", bufs=2) as outpool,
            tc.tile_pool(name="rpool", bufs=4) as rpool,
            tc.tile_pool(name="spool", bufs=3, space="PSUM") as spool,
            tc.tile_pool(name="accpool", bufs=1, space="PSUM") as accpool,
            tc.For_i(0, loop_reps, 1) if loop_reps else contextlib.nullcontext(),
        ):
            def emit_pv_block(prev, t):
                # one q-tile's full accumulation chain for the previous slot
                b_p, n_p, tin_p, pms_p, acc0_p, acc1_p = prev
                vo_p = 512 + 128 * n_p
                acc = acc0_p if t < 4 else acc1_p
                for c in range(n_p):
                    pm_t, off = pms_p[c]
                    nc.tensor.matmul(
                        acc[:, t % 4, :],
                        lhsT=pm_t[:, off + t * 128:off + (t + 1) * 128],
                        rhs=tin_p[:, vo_p + 66 * c: vo_p + 66 * c + 65],
                        start=(c == 0), stop=(c == n_p - 1),
                    )

            def emit_finish(prev):
                # normalize + store the previous slot
                b_p, n_p, tin_p, pms_p, acc0_p, acc1_p = prev
                osb = outpool.tile([128, NQT, D], F16, name="osb")
                for half, acc in ((0, acc0_p), (1, acc1_p)):
                    r = rpool.tile([128, 4], F32, tag="r", name="r")
                    nc.vector.tensor_scalar_add(
                        out=r, in0=acc[:, :, D], scalar1=1e-30
                    )
                    nc.vector.reciprocal(r, r)
                    for t4 in range(4):
                        t = half * 4 + t4
                        nc.vector.tensor_scalar_mul(
                            out=osb[:, t, :],
                            in0=acc[:, t4, 0:D],
                            scalar1=r[:, t4:t4 + 1],
                        )
                nc.sync.dma_start(out=ot.ap()[b_p], in_=osb)

            # tiny dummy exp: pulls the one-time ~2.7us ACT table load to
            # t=0 so it overlaps the first input DMA instead of serializing
            # before the first real exp
            warm = rpool.tile([128, 1], F32, tag="warm", name="warm")
            nc.vector.memset(warm, 0.0)
            nc.scalar.activation(
                out=warm, in_=warm, func=mybir.ActivationFunctionType.Exp
            )

            prev = None
            for b in range(BPC):
                sc = slot_counts[b]
                n, nz = sc if isinstance(sc, tuple) else (sc, sc)
                n = max(1, min(NCH, n))
                nz = max(1, min(n, nz))
                fz = nz - 1  # chunks 0..fz-1 are zero-bias for all batches
                used = 512 + 195 * n
                ko, bo = 512, 512 + 194 * n

                tin = inpool.tile([128, ROW], F16, tag="tin")
                nc.sync.dma_start(out=tin[:, :used], in_=inp.ap()[b][:, :used])
                qt = tin[:, 0:512]

                acc0 = accpool.tile([128, 4, D + 1], F32, tag="acc0")
                acc1 = accpool.tile([128, 4, D + 1], F32, tag="acc1")

                # Units: chunks grouped 2-per-exp on the 4-bank pair tile when
                # both are fully valid (bias identically 0 -> one fused exp
                # amortizes the ~293ns/instr ACT overhead); the partial last
                # chunk always gets its own exp with the per-partition bias.
                # Pair/single tiles strictly alternate (globally) so QK(next)
                # overlaps exp(current) with only 6 PSUM banks for scores.
                units = [(False, [c]) for c in range(n)]

                # interleave this slot's QK+exp with the previous slot's PV so
                # the in-order PE queue never parks ACT behind a PV burst
                pms = [None] * n
                nu = len(units)
                m = max(nu, NQT if prev else 0)
                pv_done = 0
                for i in range(m):
                    if prev is not None and "pv" not in ablate:
                        pv_goal = min(NQT, (NQT * (i + 1) + m - 1) // m)
                        while pv_done < pv_goal:
                            emit_pv_block(prev, pv_done)
                            pv_done += 1
                    if i < nu:
                        is_pair, chunks = units[i]
                        width = 1024 * len(chunks)
                        st = spool.tile([128, S], F32, tag="st", name="st")
                        if "qk" not in ablate:
                            for idx, c in enumerate(chunks):
                                kt = tin[:, ko + 128 * c: ko + 128 * (c + 1)]
                                o = idx * 1024
                                nc.tensor.matmul(
                                    st[:, o:o + 512],
                                    lhsT=kt[0:64, :], rhs=qt[0:64, :],
                                    start=True, stop=True,
                                )
                                nc.tensor.matmul(
                                    st[:, o + 512:o + 1024],
                                    lhsT=kt[64:128, :], rhs=qt[64:128, :],
                                    start=True, stop=True,
                                )
                        if "exp" not in ablate:
                            pm = ppool.tile([128, S], F16, tag="pm",
                                            name="pm")
                            bias = tin[:, bo + chunks[0]:bo + chunks[0] + 1]
                            nc.scalar.activation(
                                out=pm[:, :width], in_=st[:, :width],
                                func=mybir.ActivationFunctionType.Exp,
                                bias=bias, scale=0.125,
                            )
                            for idx, c in enumerate(chunks):
                                pms[c] = (pm, idx * 1024)
                if prev is not None and "pv" not in ablate:
                    while pv_done < NQT:
                        emit_pv_block(prev, pv_done)
                        pv_done += 1
                if prev is not None:
                    if "pv" not in ablate:
                        emit_finish(prev)
                    else:
                        b_p, n_p, tin_p, pms_p = prev[:4]
                        src = pms_p[-1][0] if "exp" not in ablate else tin_p
                        nc.sync.dma_start(
                            out=ot.ap()[b_p],
                            in_=src[:, 0:NQT * D].rearrange(
                                "p (t d) -> p t d", d=D
                            ),
                        )
                prev = (b, n, tin, pms, acc0, acc1)

            # drain the last slot
            if "pv" not in ablate:
                for t in range(NQT):
                    emit_pv_block(prev, t)
                emit_finish(prev)
            else:
                b_p, n_p, tin_p, pms_p = prev[:4]
                src = pms_p[-1][0] if "exp" not in ablate else tin_p
                nc.sync.dma_start(
                    out=ot.ap()[b_p],
                    in_=src[:, 0:NQT * D].rearrange("p (t d) -> p t d", d=D),
                )

    nc.compile()
    return nc


def _get_nc(slot_counts=(NCH,) * BPC):
    key = tuple(slot_counts)
    if key not in _NC_CACHE:
        _NC_CACHE[key] = _build_nc(slot_counts=key)
    return _NC_CACHE[key]


def _host_prep(queries, keys, values, valid_lens):
    queries = np.asarray(queries, dtype=np.float32)
    keys = np.asarray(keys, dtype=np.float32)
    values = np.asarray(values, dtype=np.float32)
    lens = np.asarray(valid_lens).astype(np.int64)

    q16 = queries.astype(np.float16)
    k16 = keys.astype(np.float16)
    v16 = values.astype(np.float16)

    # q halves packed into the two PE row strips: [B, 128, 512]
    qh = q16.transpose(0, 2, 1).reshape(B, 64, 2, 512)
    qh = np.ascontiguousarray(qh.transpose(0, 2, 1, 3)).reshape(B, 128, 512)

    # K^T chunks duplicated into both strips: [B, 128, NCH, 128]
    kt4 = k16.transpose(0, 2, 1).reshape(B, 64, NCH, 128)
    ktd = np.concatenate([kt4, kt4], axis=1)

    # V with ones column (pad to 66): [B, 128, NCH, 66]
    vp = np.zeros((B, 128, NCH, D + 2), np.float16)
    vp[:, :, :, :D] = v16.reshape(B, NCH, 128, D).transpose(0, 2, 1, 3)
    vp[:, :, :, D] = np.float16(1.0)

    # exp bias: 0 where k position valid, -30000 where masked: [B, 128, NCH]
    kpos = np.arange(S).reshape(NCH, 128).T  # [128, NCH] -> k = c*128 + p
    bia = np.where(
        kpos[None] < lens[:, None, None], np.float16(0.0), np.float16(-30000.0)
    ).astype(np.float16)

    # Length specialization: batch i needs ceil(L_i/128) k-chunks (min 1).
    # Sort by need, deal round-robin -> every core's slot s holds batches of
    # (near-)equal need; slot count = max within the deal group, so all cores
    # run the identical compiled program, perfectly balanced.
    need = np.maximum(1, -(-lens // 128)).astype(np.int64)
    order = np.argsort(need, kind="stable")
    gmax = [int(need[order[g * NCORES:(g + 1) * NCORES]].max()) for g in range(BPC)]
    gmin = [int(need[order[g * NCORES:(g + 1) * NCORES]].min()) for g in range(BPC)]
    perm = list(range(BPC - 1, -1, -1))  # descending: smallest slot last = tiny drain tail
    slot_counts = tuple((gmax[p], gmin[p]) for p in perm)

    in_maps = []
    for c in range(NCORES):
        fused = np.zeros((BPC, 128, ROW), np.float16)
        for s in range(BPC):
            n = slot_counts[s][0]
            b = int(order[perm[s] * NCORES + c])
            ko, vo, bo = 512, 512 + 128 * n, 512 + 194 * n
            fused[s, :, 0:512] = qh[b]
            fused[s, :, ko:ko + 128 * n] = ktd[b, :, :n].reshape(128, 128 * n)
            fused[s, :, vo:vo + 66 * n] = vp[b, :, :n, :66].reshape(128, 66 * n)
            fused[s, :, bo:bo + n] = bia[b, :, :n]
        in_maps.append({"inp": fused})
    return slot_counts, order, perm, in_maps


def kernel(queries, keys, values, valid_lens):
    slot_counts, order, perm, in_maps = _host_prep(
        queries, keys, values, valid_lens
    )
    nc = _get_nc(slot_counts)
    res = run_bass_kernel_spmd(nc, in_maps, core_ids=list(range(NCORES)))

    out = np.empty((B, S, D), np.float32)
    for c in range(NCORES):
        otv = res.results[c]["ot"]  # [BPC, 128, NQT, D] f16
        ids = [int(order[perm[s] * NCORES + c]) for s in range(BPC)]
        out[ids] = otv.transpose(0, 2, 1, 3).reshape(BPC, S, D).astype(np.float32)
    return out


# revision 39
# speedup vs baseline: 1.3694x; 1.0095x over previous
"""Masked dot-product attention (B=64, S=1024, D=64) on 8 Trainium2 NeuronCores.

Strategy (per core, 8 batches, valid-length-specialized to n k-chunks/batch):
  - One fused input DMA per batch: [Qhalf | Kt x2 | V' | bias] packed per
    partition (DMA latency is 650ns/issue; traffic is the measured floor).
  - S^T chunks [k=128, q=1024] = K_chunk @ Q^T on PE, D=64 contraction on
    partitions; the two 64-row strips of the PE array compute the two q-halves
    of the SAME chunk concurrently (tile_position row packing, no Q dup).
  - exp via ACT, per-partition bias 0/-30000 folds the valid_lens mask into
    the softmax; 1/sqrt(D) folded into the ACT scale. P^T in fp16.
  - P @ [V | 1]: P^T slices stationary; column 64 of the accumulator is the
    softmax denominator. normalize = reciprocal + tensor_scalar_mul -> fp16
    out, upcast to fp32 on host.
Host does layout prep only (transpose/cast/pack/shard) - all FLOPs on device.
"""

import contextlib

import numpy as np

import concourse.bass as bass  # noqa: F401
import concourse.bacc as bacc
import concourse.mybir as mybir
import concourse.tile as tile
from concourse.bass_utils import run_bass_kernel_spmd

B, S, D = 64, 1024, 64
NCORES = 8
BPC = B // NCORES          # batches per core
NCH = S // 128             # k chunks of 128
NQT = S // 128             # q tiles of 128
F16 = mybir.dt.float16
F32 = mybir.dt.float32

# fused input row layout (f16 elements per partition):
#   [0:512)                q half (strip 0: q 0-511, strip 1: q 512-1023)
#   [512 : 512+128n)       K^T chunk c at 512+128c (same data in both strips)
#   [+ : +66n)             V' chunk c at +66c (65 used + 1 pad)
#   [+ : +n)               exp bias per chunk (0 / -30000, f16)
ROW = 512 + 195 * NCH  # 2072

# DVE-exp offload (Schraudolph bit-trick, quadratic 2^f, ~0.2% max err):
# route every DVE_EVERY-th compile-time-fully-valid chunk to the otherwise
# idle vector engine to shave the ACT exp roofline.
DVE_EVERY = 0  # disabled: measured 9-op DVE exp chain runs ~1us/op (no 2x
# mode for fp32 tensor_scalar) -> DVE becomes the bottleneck. Kept for reference.
_EXP_C0 = 0.18033688011112042  # 0.125 * log2(e)
_EXP_MAGIC = 12582912.0        # 1.5 * 2^23: float add rounds y to int
_EXP_A = 1.475187301175261
_EXP_B = 0.23842257574160022
_EXP_C = 0.48159279147134226

_NC_CACHE = {}


def _build_nc(loop_reps=None, slot_counts=(NCH,) * BPC, ablate=frozenset()):
    # slot_counts entries: n (chunks to compute) or (n, nz) where chunks
    # 0..nz-2 are fully valid for EVERY batch dealt into that slot (group min
    # need) and may share a fused zero-bias exp; the rest always read their
    # per-chunk bias vector.
    nc = bacc.Bacc(None, target_bir_lowering=False)
    inp = nc.dram_tensor("inp", [BPC, 128, ROW], F16, kind="ExternalInput")
    ot = nc.dram_tensor("ot", [BPC, 128, NQT, D], F16, kind="ExternalOutput")

    with tile.TileContext(nc) as tc:
        with (
            tc.tile_pool(name="inpool", bufs=3) as inpool,
            tc.tile_pool(name="ppool", bufs=18) as ppool,
            tc.tile_pool(name="outpool", bufs=2) as outpool,
            tc.tile_pool(name="rpool", bufs=4) as rpool,
            tc.tile_pool(name="dvp", bufs=2) as dvp,
            tc.tile_pool(name="spool", bufs=3, space="PSUM") as spool,
            tc.tile_pool(name="accpool", bufs=1, space="PSUM") as accpool,
            tc.For_i(0, loop_reps, 1) if loop_reps else contextlib.nullcontext(),
        ):
            def emit_pv_block(prev, t):
                # one q-tile's full accumulation chain for the previous slot
                b_p, n_p, tin_p, pms_p, acc0_p, acc1_p = prev
                vo_p = 512 + 128 * n_p
                acc = acc0_p if t < 4 else acc1_p
                for c in range(n_p):
                    pm_t, off = pms_p[c]
                    nc.tensor.matmul(
                        acc[:, t % 4, :],
                        lhsT=pm_t[:, off + t * 128:off + (t + 1) * 128],
                        rhs=tin_p[:, vo_p + 66 * c: vo_p + 66 * c + 65],
                        start=(c == 0), stop=(c == n_p - 1),
                    )

            def emit_dve_exp(st, pm, dvp):
                I32 = mybir.dt.int32
                T = dvp.tile([128, S], F32, tag="dt", name="dt")
                nc.vector.tensor_scalar(
                    out=T, in0=st, scalar1=_EXP_C0, scalar2=_EXP_MAGIC,
                    op0=mybir.AluOpType.mult, op1=mybir.AluOpType.add)
                U = dvp.tile([128, S], F32, tag="du", name="du")
                nc.vector.tensor_scalar_add(out=U, in0=T, scalar1=-_EXP_MAGIC)
                Fq = dvp.tile([128, S], F32, tag="df", name="df")
                nc.vector.scalar_tensor_tensor(
                    out=Fq, in0=st, in1=U, scalar=_EXP_C0,
                    op0=mybir.AluOpType.mult, op1=mybir.AluOpType.subtract)
                G = dvp.tile([128, S], F32, tag="dg", name="dg")
                nc.vector.tensor_scalar_add(out=G, in0=Fq, scalar1=_EXP_A)
                H = dvp.tile([128, S], F32, tag="dh", name="dh")
                nc.vector.tensor_mul(out=H, in0=G, in1=G)
                P_ = dvp.tile([128, S], F32, tag="dp", name="dp")
                nc.vector.tensor_scalar(
                    out=P_, in0=H, scalar1=_EXP_B, scalar2=_EXP_C,
                    op0=mybir.AluOpType.mult, op1=mybir.AluOpType.add)
                SH = dvp.tile([128, S], I32, tag="dsh", name="dsh")
                nc.vector.tensor_scalar(
                    out=SH, in0=T.bitcast(I32), scalar1=23, scalar2=None,
                    op0=mybir.AluOpType.logical_shift_left)
                R = dvp.tile([128, S], I32, tag="dr", name="dr")
                nc.vector.tensor_add(out=R, in0=P_.bitcast(I32), in1=SH)
                nc.vector.tensor_copy(out=pm, in_=R.bitcast(F32))

            def emit_finish(prev):
                # normalize + store the previous slot
                b_p, n_p, tin_p, pms_p, acc0_p, acc1_p = prev
                osb = outpool.tile([128, NQT, D], F16, name="osb")
                for half, acc in ((0, acc0_p), (1, acc1_p)):
                    r = rpool.tile([128, 4], F32, tag="r", name="r")
                    nc.vector.tensor_scalar_add(
                        out=r, in0=acc[:, :, D], scalar1=1e-30
                    )
                    nc.vector.reciprocal(r, r)
                    for t4 in range(4):
                        t = half * 4 + t4
                        nc.vector.tensor_scalar_mul(
                            out=osb[:, t, :],
                            in0=acc[:, t4, 0:D],
                            scalar1=r[:, t4:t4 + 1],
                        )
                nc.sync.dma_start(out=ot.ap()[b_p], in_=osb)

            # tiny dummy exp: pulls the one-time ~2.7us ACT table load to
            # t=0 so it overlaps the first input DMA instead of serializing
            # before the first real exp
            warm = rpool.tile([128, 1], F32, tag="warm", name="warm")
            nc.vector.memset(warm, 0.0)
            nc.scalar.activation(
                out=warm, in_=warm, func=mybir.ActivationFunctionType.Exp
            )

            prev = None
            full_ctr = [0]
            toggle = [True]
            for b in range(BPC):
                sc = slot_counts[b]
                n, nz = sc if isinstance(sc, tuple) else (sc, sc)
                n = max(1, min(NCH, n))
                nz = max(1, min(n, nz))
                fz = nz - 1  # chunks 0..fz-1 are zero-bias for all batches
                used = 512 + 195 * n
                ko, bo = 512, 512 + 194 * n

                tin = inpool.tile([128, ROW], F16, tag="tin")
                nc.sync.dma_start(out=tin[:, :used], in_=inp.ap()[b][:, :used])
                qt = tin[:, 0:512]

                acc0 = accpool.tile([128, 4, D + 1], F32, tag="acc0")
                acc1 = accpool.tile([128, 4, D + 1], F32, tag="acc1")

                # Units: chunks grouped 2-per-exp on the 4-bank pair tile when
                # both are fully valid (bias identically 0 -> one fused exp
                # amortizes the ~293ns/instr ACT overhead); the partial last
                # chunk always gets its own exp with the per-partition bias.
                # Pair/single tiles strictly alternate (globally) so QK(next)
                # overlaps exp(current) with only 6 PSUM banks for scores.
                units = [(False, [c]) for c in range(n)]

                # interleave this slot's QK+exp with the previous slot's PV so
                # the in-order PE queue never parks ACT behind a PV burst
                pms = [None] * n
                nu = len(units)
                m = max(nu, NQT if prev else 0)
                pv_done = 0
                for i in range(m):
                    if prev is not None and "pv" not in ablate:
                        pv_goal = min(NQT, (NQT * (i + 1) + m - 1) // m)
                        while pv_done < pv_goal:
                            emit_pv_block(prev, pv_done)
                            pv_done += 1
                    if i < nu:
                        is_pair, chunks = units[i]
                        width = 1024 * len(chunks)
                        st = spool.tile([128, S], F32, tag="st", name="st")
                        if "qk" not in ablate:
                            for idx, c in enumerate(chunks):
                                kt = tin[:, ko + 128 * c: ko + 128 * (c + 1)]
                                o = idx * 1024
                                nc.tensor.matmul(
                                    st[:, o:o + 512],
                                    lhsT=kt[0:64, :], rhs=qt[0:64, :],
                                    start=True, stop=True,
                                )
                                nc.tensor.matmul(
                                    st[:, o + 512:o + 1024],
                                    lhsT=kt[64:128, :], rhs=qt[64:128, :],
                                    start=True, stop=True,
                                )
                        if "exp" not in ablate:
                            pm = ppool.tile([128, S], F16, tag="pm",
                                            name="pm")
                            c0 = chunks[0]
                            on_dve = False
                            if DVE_EVERY and c0 < fz:
                                on_dve = full_ctr[0] % DVE_EVERY == 1
                                full_ctr[0] += 1
                            if on_dve:
                                emit_dve_exp(st, pm, dvp)
                            else:
                                bias = tin[:, bo + c0:bo + c0 + 1]
                                nc.scalar.activation(
                                    out=pm[:, :width], in_=st[:, :width],
                                    func=mybir.ActivationFunctionType.Exp,
                                    bias=bias, scale=0.125,
                                )
                            for idx, c in enumerate(chunks):
                                pms[c] = (pm, idx * 1024)
                if prev is not None and "pv" not in ablate:
                    while pv_done < NQT:
                        emit_pv_block(prev, pv_done)
                        pv_done += 1
                if prev is not None:
                    if "pv" not in ablate:
                        emit_finish(prev)
                    else:
                        b_p, n_p, tin_p, pms_p = prev[:4]
                        src = pms_p[-1][0] if "exp" not in ablate else tin_p
                        nc.sync.dma_start(
                            out=ot.ap()[b_p],
                            in_=src[:, 0:NQT * D].rearrange(
                                "p (t d) -> p t d", d=D
                            ),
                        )
                prev = (b, n, tin, pms, acc0, acc1)

            # drain the last slot
            if "pv" not in ablate:
                for t in range(NQT):
                    emit_pv_block(prev, t)
                emit_finish(prev)
            else:
                b_p, n_p, tin_p, pms_p = prev[:4]
                src = pms_p[-1][0] if "exp" not in ablate else tin_p
                nc.sync.dma_start(
                    out=ot.ap()[b_p],
                    in_=src[:, 0:NQT * D].rearrange("p (t d) -> p t d", d=D),
                )

    nc.compile()
    return nc


def _get_nc(slot_counts=(NCH,) * BPC):
    key = tuple(slot_counts)
    if key not in _NC_CACHE:
        _NC_CACHE[key] = _build_nc(slot_counts=key)
    return _NC_CACHE[key]


def _host_prep(queries, keys, values, valid_lens):
    queries = np.asarray(queries, dtype=np.float32)
    keys = np.asarray(keys, dtype=np.float32)
    values = np.asarray(values, dtype=np.float32)
    lens = np.asarray(valid_lens).astype(np.int64)

    q16 = queries.astype(np.float16)
    k16 = keys.astype(np.float16)
    v16 = values.astype(np.float16)

    # q halves packed into the two PE row strips: [B, 128, 512]
    qh = q16.transpose(0, 2, 1).reshape(B, 64, 2, 512)
    qh = np.ascontiguousarray(qh.transpose(0, 2, 1, 3)).reshape(B, 128, 512)

    # K^T chunks duplicated into both strips: [B, 128, NCH, 128]
    kt4 = k16.transpose(0, 2, 1).reshape(B, 64, NCH, 128)
    ktd = np.concatenate([kt4, kt4], axis=1)

    # V with ones column (pad to 66): [B, 128, NCH, 66]
    vp = np.zeros((B, 128, NCH, D + 2), np.float16)
    vp[:, :, :, :D] = v16.reshape(B, NCH, 128, D).transpose(0, 2, 1, 3)
    vp[:, :, :, D] = np.float16(1.0)

    # exp bias: 0 where k position valid, -30000 where masked: [B, 128, NCH]
    kpos = np.arange(S).reshape(NCH, 128).T  # [128, NCH] -> k = c*128 + p
    bia = np.where(
        kpos[None] < lens[:, None, None], np.float16(0.0), np.float16(-30000.0)
    ).astype(np.float16)

    # Length specialization: batch i needs ceil(L_i/128) k-chunks (min 1).
    # Sort by need, deal round-robin -> every core's slot s holds batches of
    # (near-)equal need; slot count = max within the deal group, so all cores
    # run the identical compiled program, perfectly balanced.
    need = np.maximum(1, -(-lens // 128)).astype(np.int64)
    order = np.argsort(need, kind="stable")
    gmax = [int(need[order[g * NCORES:(g + 1) * NCORES]].max()) for g in range(BPC)]
    gmin = [int(need[order[g * NCORES:(g + 1) * NCORES]].min()) for g in range(BPC)]
    perm = list(range(BPC - 1, -1, -1))  # descending: smallest slot last = tiny drain tail
    slot_counts = tuple((gmax[p], gmin[p]) for p in perm)

    in_maps = []
    for c in range(NCORES):
        fused = np.zeros((BPC, 128, ROW), np.float16)
        for s in range(BPC):
            n = slot_counts[s][0]
            b = int(order[perm[s] * NCORES + c])
            ko, vo, bo = 512, 512 + 128 * n, 512 + 194 * n
            fused[s, :, 0:512] = qh[b]
            fused[s, :, ko:ko + 128 * n] = ktd[b, :, :n].reshape(128, 128 * n)
            fused[s, :, vo:vo + 66 * n] = vp[b, :, :n, :66].reshape(128, 66 * n)
            fused[s, :, bo:bo + n] = bia[b, :, :n]
        in_maps.append({"inp": fused})
    return slot_counts, order, perm, in_maps


def kernel(queries, keys, values, valid_lens):
    slot_counts, order, perm, in_maps = _host_prep(
        queries, keys, values, valid_lens
    )
    nc = _get_nc(slot_counts)
    res = run_bass_kernel_spmd(nc, in_maps, core_ids=list(range(NCORES)))

    out = np.empty((B, S, D), np.float32)
    for c in range(NCORES):
        otv = res.results[c]["ot"]  # [BPC, 128, NQT, D] f16
        ids = [int(order[perm[s] * NCORES + c]) for s in range(BPC)]
        out[ids] = otv.transpose(0, 2, 1, 3).reshape(BPC, S, D).astype(np.float32)
    return out


# revision 43
# speedup vs baseline: 1.3994x; 1.0219x over previous
"""Masked dot-product attention (B=64, S=1024, D=64) on 8 Trainium2 NeuronCores.

Strategy (per core, 8 batches, valid-length-specialized to n k-chunks/batch):
  - Two fused input DMAs per batch: head tile [Qhalf | bias | Kt chunk0]
    (1.3KB/partition, unblocks QK+exp immediately) and bulk tile
    [Kt chunks 1.. | V'] that only gates later chunks and PV.
  - S^T chunks [k=128, q=1024] = K_chunk @ Q^T on PE, D=64 contraction on
    partitions; the two 64-row strips of the PE array compute the two q-halves
    of the SAME chunk concurrently (tile_position row packing, no Q dup).
  - exp via ACT, per-partition bias 0/-30000 folds the valid_lens mask into
    the softmax; 1/sqrt(D) folded into the ACT scale. P^T in fp16.
  - P @ [V | 1]: P^T slices stationary; column 64 of the accumulator is the
    softmax denominator. normalize = reciprocal + tensor_scalar_mul -> fp16
    out, upcast to fp32 on host.
Host does layout prep only (transpose/cast/pack/shard) - all FLOPs on device.
"""

import contextlib

import numpy as np

import concourse.bass as bass  # noqa: F401
import concourse.bacc as bacc
import concourse.mybir as mybir
import concourse.tile as tile
from concourse.bass_utils import run_bass_kernel_spmd

B, S, D = 64, 1024, 64
NCORES = 8
BPC = B // NCORES          # batches per core
NCH = S // 128             # k chunks of 128
NQT = S // 128             # q tiles of 128
F16 = mybir.dt.float16
F32 = mybir.dt.float32

# fused input row layout (f16 elements per partition):
#   [0:512)                q half (strip 0: q 0-511, strip 1: q 512-1023)
#   [512 : 512+128n)       K^T chunk c at 512+128c (same data in both strips)
#   [+ : +66n)             V' chunk c at +66c (65 used + 1 pad)
#   [+ : +n)               exp bias per chunk (0 / -30000, f16)
ROW = 512 + 195 * NCH  # 2072 (legacy single-row length, kept for bench scripts)
# split input rows: head tile unblocks QK+exp after ~1.3KB/partition;
# bulk tile (kt chunks 1.. + V') only gates later chunks and PV
AROW = 512 + NCH + 128        # [qt | bias(NCH) | kt chunk0]
BROW = 128 * (NCH - 1) + 66 * NCH  # [kt chunks 1.. | vv]

# DVE-exp offload (Schraudolph bit-trick, quadratic 2^f, ~0.2% max err):
# route every DVE_EVERY-th compile-time-fully-valid chunk to the otherwise
# idle vector engine to shave the ACT exp roofline.
DVE_EVERY = 0  # disabled: measured 9-op DVE exp chain runs ~1us/op (no 2x
# mode for fp32 tensor_scalar) -> DVE becomes the bottleneck. Kept for reference.
_EXP_C0 = 0.18033688011112042  # 0.125 * log2(e)
_EXP_MAGIC = 12582912.0        # 1.5 * 2^23: float add rounds y to int
_EXP_A = 1.475187301175261
_EXP_B = 0.23842257574160022
_EXP_C = 0.48159279147134226

_NC_CACHE = {}


def _build_nc(loop_reps=None, slot_counts=(NCH,) * BPC, ablate=frozenset()):
    # slot_counts entries: n (chunks to compute) or (n, nz) where chunks
    # 0..nz-2 are fully valid for EVERY batch dealt into that slot (group min
    # need) and may share a fused zero-bias exp; the rest always read their
    # per-chunk bias vector.
    nc = bacc.Bacc(None, target_bir_lowering=False)
    inpa = nc.dram_tensor("inpa", [BPC, 128, AROW], F16, kind="ExternalInput")
    inpb = nc.dram_tensor("inpb", [BPC, 128, BROW], F16, kind="ExternalInput")
    ot = nc.dram_tensor("ot", [BPC, 128, NQT, D], F16, kind="ExternalOutput")

    with tile.TileContext(nc) as tc:
        with (
            tc.tile_pool(name="inpool", bufs=3) as inpool,
            tc.tile_pool(name="ppool", bufs=18) as ppool,
            tc.tile_pool(name="outpool", bufs=2) as outpool,
            tc.tile_pool(name="rpool", bufs=4) as rpool,
            tc.tile_pool(name="dvp", bufs=2) as dvp,
            tc.tile_pool(name="spool", bufs=3, space="PSUM") as spool,
            tc.tile_pool(name="accpool", bufs=1, space="PSUM") as accpool,
            tc.For_i(0, loop_reps, 1) if loop_reps else contextlib.nullcontext(),
        ):
            def emit_pv_block(prev, t):
                # one q-tile's full accumulation chain for the previous slot
                b_p, n_p, tb_p, pms_p, acc0_p, acc1_p = prev
                vo_p = 128 * (n_p - 1)
                acc = acc0_p if t < 4 else acc1_p
                for c in range(n_p):
                    pm_t, off = pms_p[c]
                    nc.tensor.matmul(
                        acc[:, t % 4, :],
                        lhsT=pm_t[:, off + t * 128:off + (t + 1) * 128],
                        rhs=tb_p[:, vo_p + 66 * c: vo_p + 66 * c + 65],
                        start=(c == 0), stop=(c == n_p - 1),
                    )

            def emit_dve_exp(st, pm, dvp):
                I32 = mybir.dt.int32
                T = dvp.tile([128, S], F32, tag="dt", name="dt")
                nc.vector.tensor_scalar(
                    out=T, in0=st, scalar1=_EXP_C0, scalar2=_EXP_MAGIC,
                    op0=mybir.AluOpType.mult, op1=mybir.AluOpType.add)
                U = dvp.tile([128, S], F32, tag="du", name="du")
                nc.vector.tensor_scalar_add(out=U, in0=T, scalar1=-_EXP_MAGIC)
                Fq = dvp.tile([128, S], F32, tag="df", name="df")
                nc.vector.scalar_tensor_tensor(
                    out=Fq, in0=st, in1=U, scalar=_EXP_C0,
                    op0=mybir.AluOpType.mult, op1=mybir.AluOpType.subtract)
                G = dvp.tile([128, S], F32, tag="dg", name="dg")
                nc.vector.tensor_scalar_add(out=G, in0=Fq, scalar1=_EXP_A)
                H = dvp.tile([128, S], F32, tag="dh", name="dh")
                nc.vector.tensor_mul(out=H, in0=G, in1=G)
                P_ = dvp.tile([128, S], F32, tag="dp", name="dp")
                nc.vector.tensor_scalar(
                    out=P_, in0=H, scalar1=_EXP_B, scalar2=_EXP_C,
                    op0=mybir.AluOpType.mult, op1=mybir.AluOpType.add)
                SH = dvp.tile([128, S], I32, tag="dsh", name="dsh")
                nc.vector.tensor_scalar(
                    out=SH, in0=T.bitcast(I32), scalar1=23, scalar2=None,
                    op0=mybir.AluOpType.logical_shift_left)
                R = dvp.tile([128, S], I32, tag="dr", name="dr")
                nc.vector.tensor_add(out=R, in0=P_.bitcast(I32), in1=SH)
                nc.vector.tensor_copy(out=pm, in_=R.bitcast(F32))

            def emit_finish_half(prev, half, osb):
                # normalize + store one 4-q-tile half of the previous slot
                b_p, n_p, tb_p, pms_p, acc0_p, acc1_p = prev
                acc = acc0_p if half == 0 else acc1_p
                r = rpool.tile([128, 4], F32, tag="r", name="r")
                nc.vector.tensor_scalar_add(
                    out=r, in0=acc[:, :, D], scalar1=1e-30
                )
                nc.vector.reciprocal(r, r)
                for t4 in range(4):
                    t = half * 4 + t4
                    nc.vector.tensor_scalar_mul(
                        out=osb[:, t, :],
                        in0=acc[:, t4, 0:D],
                        scalar1=r[:, t4:t4 + 1],
                    )
                nc.sync.dma_start(
                    out=ot.ap()[b_p][:, half * 4:(half + 1) * 4, :],
                    in_=osb[:, half * 4:(half + 1) * 4, :],
                )

            def emit_finish(prev):
                osb = outpool.tile([128, NQT, D], F16, name="osb")
                emit_finish_half(prev, 0, osb)
                emit_finish_half(prev, 1, osb)

            # tiny dummy exp: pulls the one-time ~2.7us ACT table load to
            # t=0 so it overlaps the first input DMA instead of serializing
            # before the first real exp
            warm = rpool.tile([128, 1], F32, tag="warm", name="warm")
            nc.vector.memset(warm, 0.0)
            nc.scalar.activation(
                out=warm, in_=warm, func=mybir.ActivationFunctionType.Exp
            )

            prev = None
            full_ctr = [0]
            toggle = [True]
            for b in range(BPC):
                sc = slot_counts[b]
                n, nz = sc if isinstance(sc, tuple) else (sc, sc)
                n = max(1, min(NCH, n))
                nz = max(1, min(n, nz))
                fz = nz - 1  # chunks 0..fz-1 are zero-bias for all batches
                ua = 512 + n + 128
                ub = 128 * (n - 1) + 66 * n

                ta = inpool.tile([128, AROW], F16, tag="ta", name="ta")
                nc.sync.dma_start(out=ta[:, :ua], in_=inpa.ap()[b][:, :ua])
                tb = inpool.tile([128, BROW], F16, tag="tb", name="tb")
                nc.sync.dma_start(out=tb[:, :ub], in_=inpb.ap()[b][:, :ub])
                qt = ta[:, 0:512]

                acc0 = accpool.tile([128, 4, D + 1], F32, tag="acc0")
                acc1 = accpool.tile([128, 4, D + 1], F32, tag="acc1")

                # Units: chunks grouped 2-per-exp on the 4-bank pair tile when
                # both are fully valid (bias identically 0 -> one fused exp
                # amortizes the ~293ns/instr ACT overhead); the partial last
                # chunk always gets its own exp with the per-partition bias.
                # Pair/single tiles strictly alternate (globally) so QK(next)
                # overlaps exp(current) with only 6 PSUM banks for scores.
                units = [(False, [c]) for c in range(n)]

                # interleave this slot's QK+exp with the previous slot's PV so
                # the in-order PE queue never parks ACT behind a PV burst
                pms = [None] * n
                nu = len(units)
                m = max(nu, NQT if prev else 0)
                pv_done = 0
                for i in range(m):
                    if prev is not None and "pv" not in ablate:
                        pv_goal = min(NQT, (NQT * (i + 1) + m - 1) // m)
                        while pv_done < pv_goal:
                            emit_pv_block(prev, pv_done)
                            pv_done += 1
                    if i < nu:
                        is_pair, chunks = units[i]
                        width = 1024 * len(chunks)
                        st = spool.tile([128, S], F32, tag="st", name="st")
                        if "qk" not in ablate:
                            for idx, c in enumerate(chunks):
                                kt = (
                                    ta[:, 512 + n:512 + n + 128] if c == 0
                                    else tb[:, 128 * (c - 1):128 * c]
                                )
                                o = idx * 1024
                                nc.tensor.matmul(
                                    st[:, o:o + 512],
                                    lhsT=kt[0:64, :], rhs=qt[0:64, :],
                                    start=True, stop=True,
                                )
                                nc.tensor.matmul(
                                    st[:, o + 512:o + 1024],
                                    lhsT=kt[64:128, :], rhs=qt[64:128, :],
                                    start=True, stop=True,
                                )
                        if "exp" not in ablate:
                            pm = ppool.tile([128, S], F16, tag="pm",
                                            name="pm")
                            c0 = chunks[0]
                            on_dve = False
                            if DVE_EVERY and c0 < fz:
                                on_dve = full_ctr[0] % DVE_EVERY == 1
                                full_ctr[0] += 1
                            if on_dve:
                                emit_dve_exp(st, pm, dvp)
                            else:
                                bias = ta[:, 512 + c0:512 + c0 + 1]
                                nc.scalar.activation(
                                    out=pm[:, :width], in_=st[:, :width],
                                    func=mybir.ActivationFunctionType.Exp,
                                    bias=bias, scale=0.125,
                                )
                            for idx, c in enumerate(chunks):
                                pms[c] = (pm, idx * 1024)
                if prev is not None and "pv" not in ablate:
                    while pv_done < NQT:
                        emit_pv_block(prev, pv_done)
                        pv_done += 1
                if prev is not None:
                    if "pv" not in ablate:
                        emit_finish(prev)
                    else:
                        b_p, n_p, tb_p, pms_p = prev[:4]
                        src = pms_p[-1][0] if "exp" not in ablate else tb_p
                        nc.sync.dma_start(
                            out=ot.ap()[b_p],
                            in_=src[:, 0:NQT * D].rearrange(
                                "p (t d) -> p t d", d=D
                            ),
                        )
                prev = (b, n, tb, pms, acc0, acc1)

            # drain the last slot: finish+store half 0 while half 1's PV runs
            if "pv" not in ablate:
                osb = outpool.tile([128, NQT, D], F16, name="osb")
                for t in range(NQT):
                    emit_pv_block(prev, t)
                    if t == 3:
                        emit_finish_half(prev, 0, osb)
                emit_finish_half(prev, 1, osb)
            else:
                b_p, n_p, tin_p, pms_p = prev[:4]
                src = pms_p[-1][0] if "exp" not in ablate else tin_p
                nc.sync.dma_start(
                    out=ot.ap()[b_p],
                    in_=src[:, 0:NQT * D].rearrange("p (t d) -> p t d", d=D),
                )

    nc.compile()
    return nc


def _get_nc(slot_counts=(NCH,) * BPC):
    key = tuple(slot_counts)
    if key not in _NC_CACHE:
        _NC_CACHE[key] = _build_nc(slot_counts=key)
    return _NC_CACHE[key]


def _host_prep(queries, keys, values, valid_lens):
    queries = np.asarray(queries, dtype=np.float32)
    keys = np.asarray(keys, dtype=np.float32)
    values = np.asarray(values, dtype=np.float32)
    lens = np.asarray(valid_lens).astype(np.int64)

    q16 = queries.astype(np.float16)
    k16 = keys.astype(np.float16)
    v16 = values.astype(np.float16)

    # q halves packed into the two PE row strips: [B, 128, 512]
    qh = q16.transpose(0, 2, 1).reshape(B, 64, 2, 512)
    qh = np.ascontiguousarray(qh.transpose(0, 2, 1, 3)).reshape(B, 128, 512)

    # K^T chunks duplicated into both strips: [B, 128, NCH, 128]
    kt4 = k16.transpose(0, 2, 1).reshape(B, 64, NCH, 128)
    ktd = np.concatenate([kt4, kt4], axis=1)

    # V with ones column (pad to 66): [B, 128, NCH, 66]
    vp = np.zeros((B, 128, NCH, D + 2), np.float16)
    vp[:, :, :, :D] = v16.reshape(B, NCH, 128, D).transpose(0, 2, 1, 3)
    vp[:, :, :, D] = np.float16(1.0)

    # exp bias: 0 where k position valid, -30000 where masked: [B, 128, NCH]
    kpos = np.arange(S).reshape(NCH, 128).T  # [128, NCH] -> k = c*128 + p
    bia = np.where(
        kpos[None] < lens[:, None, None], np.float16(0.0), np.float16(-30000.0)
    ).astype(np.float16)

    # Length specialization: batch i needs ceil(L_i/128) k-chunks (min 1).
    # Sort by need, deal round-robin -> every core's slot s holds batches of
    # (near-)equal need; slot count = max within the deal group, so all cores
    # run the identical compiled program, perfectly balanced.
    need = np.maximum(1, -(-lens // 128)).astype(np.int64)
    order = np.argsort(need, kind="stable")
    gmax = [int(need[order[g * NCORES:(g + 1) * NCORES]].max()) for g in range(BPC)]
    gmin = [int(need[order[g * NCORES:(g + 1) * NCORES]].min()) for g in range(BPC)]
    perm = list(range(BPC - 1, -1, -1))  # descending: smallest slot last = tiny drain tail
    slot_counts = tuple((gmax[p], gmin[p]) for p in perm)

    in_maps = []
    for c in range(NCORES):
        fa = np.zeros((BPC, 128, AROW), np.float16)
        fb = np.zeros((BPC, 128, BROW), np.float16)
        for s in range(BPC):
            n = slot_counts[s][0]
            b = int(order[perm[s] * NCORES + c])
            fa[s, :, 0:512] = qh[b]
            fa[s, :, 512:512 + n] = bia[b, :, :n]
            fa[s, :, 512 + n:512 + n + 128] = ktd[b, :, 0]
            if n > 1:
                fb[s, :, :128 * (n - 1)] = (
                    ktd[b, :, 1:n].reshape(128, 128 * (n - 1))
                )
            vo = 128 * (n - 1)
            fb[s, :, vo:vo + 66 * n] = vp[b, :, :n, :66].reshape(128, 66 * n)
        in_maps.append({"inpa": fa, "inpb": fb})
    return slot_counts, order, perm, in_maps


def kernel(queries, keys, values, valid_lens):
    slot_counts, order, perm, in_maps = _host_prep(
        queries, keys, values, valid_lens
    )
    nc = _get_nc(slot_counts)
    res = run_bass_kernel_spmd(nc, in_maps, core_ids=list(range(NCORES)))

    out = np.empty((B, S, D), np.float32)
    for c in range(NCORES):
        otv = res.results[c]["ot"]  # [BPC, 128, NQT, D] f16
        ids = [int(order[perm[s] * NCORES + c]) for s in range(BPC)]
        out[ids] = otv.transpose(0, 2, 1, 3).reshape(BPC, S, D).astype(np.float32)
    return out
